# revision 3
# baseline (speedup 1.0000x reference)
"""Causal scaled-dot-product attention for Trainium2 (Bass/Tile), 8-core SPMD.

Problem: B=2, H=16, S=2048, D=128 fp32, causal mask, softmax(QK^T/sqrt(D)) @ V.
Sharding: batch*heads (32) split across 8 cores, 4 heads per core. Attention is
independent per (b,h): no communication.

Per-head algorithm (S^T layout — avoids any transpose of the probability
matrix):
  - PE-transpose Q,K once -> Q^T,K^T  [d=128 partitions, seq free]
  - for each 512-wide query chunk c:
      for each key tile j (128 keys) at or below the diagonal:
        S^T[j] = K_j @ Q_c^T          (fp32r matmul, PSUM)
        P^T[j] = exp(S^T[j] / temp)   (ACT, PSUM->SBUF, f32r)
        diagonal tiles masked with an upper-triangular constant
        OUT^T  += V_j^T @ P^T[j]      (fp32r matmul, V in natural layout)
        den    += ones^T @ P^T[j]     (fp32r matmul, [1, 512])
      OUT = transpose(OUT^T * (1/den)) -> DRAM
Softmax max-subtraction is skipped: logits are bounded (~20) so exp is safe in
fp32, and softmax is shift-invariant.
"""
import numpy as np

import concourse.bacc as bacc
import concourse.tile as tile
import concourse.mybir as mybir
from concourse.bass_utils import run_bass_kernel_spmd
from concourse.masks import make_identity, make_upper_triangular

F32 = mybir.dt.float32
F32R = mybir.dt.float32r
EXP = mybir.ActivationFunctionType.Exp

B, H, S, D = 2, 16, 2048, 128
TEMPERATURE = 11.313708498984761  # sqrt(128)
N_CORES = 8
HEADS_PER_CORE = (B * H) // N_CORES  # 4
P = 128                    # partitions / tile edge
CHUNK = 512                # query chunk (1 PSUM bank of fp32)
N_KT = S // P              # 16 key tiles per head
N_CH = S // CHUNK          # 4 query chunks per head


def build_attention_nc(rep=1):
    nc = bacc.Bacc("TRN2", target_bir_lowering=False, debug=False,
                   num_devices=N_CORES)
    q_d = nc.dram_tensor("q", [HEADS_PER_CORE, S, D], F32, kind="ExternalInput").ap()
    k_d = nc.dram_tensor("k", [HEADS_PER_CORE, S, D], F32, kind="ExternalInput").ap()
    v_d = nc.dram_tensor("v", [HEADS_PER_CORE, S, D], F32, kind="ExternalInput").ap()
    o_d = nc.dram_tensor("out", [HEADS_PER_CORE, S, D], F32, kind="ExternalOutput").ap()

    with tile.TileContext(nc) as tc:
        with tc.tile_pool(name="consts", bufs=1) as consts, \
             tc.tile_pool(name="inb", bufs=2) as inb, \
             tc.tile_pool(name="qkt", bufs=2) as qkt, \
             tc.tile_pool(name="px", bufs=4) as px, \
             tc.tile_pool(name="sm", bufs=3) as sm, \
             tc.tile_pool(name="ps_s", bufs=2, space="PSUM") as ps_s, \
             tc.tile_pool(name="ps_o", bufs=2, space="PSUM") as ps_o, \
             tc.tile_pool(name="ps_d", bufs=1, space="PSUM") as ps_d, \
             tc.tile_pool(name="ps_t", bufs=1, space="PSUM") as ps_t:

            # ---- constants ----
            ident = consts.tile([P, P], F32)
            make_identity(nc, ident)
            ident_r = consts.tile([P, P], F32R)
            nc.vector.tensor_copy(ident_r, ident)
            utm = consts.tile([P, P], F32)  # utm[k,q] = 1 iff q >= k
            make_upper_triangular(nc, utm, val=1.0, diag=True)
            ones_f = consts.tile([P, 1], F32)
            nc.vector.memset(ones_f, 1.0)
            ones_col = consts.tile([P, 1], F32R)
            nc.vector.tensor_copy(ones_col, ones_f)

            for h_rep in range(rep * HEADS_PER_CORE):
                h = h_rep % HEADS_PER_CORE
                # ---- load (cast fp32 -> f32r during DMA) ----
                qn = inb.tile([P, N_KT, P], F32R, tag="qn")
                kn = inb.tile([P, N_KT, P], F32R, tag="kn")
                vn = inb.tile([P, N_KT, P], F32R, tag="vn")
                nc.gpsimd.dma_start(
                    out=qn, in_=q_d[h].rearrange("(t p) d -> p t d", p=P))
                nc.gpsimd.dma_start(
                    out=kn, in_=k_d[h].rearrange("(t p) d -> p t d", p=P))
                nc.gpsimd.dma_start(
                    out=vn, in_=v_d[h].rearrange("(t p) d -> p t d", p=P))

                # ---- transpose Q,K -> [d, seq] ----
                qT = qkt.tile([P, S], F32R, tag="qT")
                kT = qkt.tile([P, S], F32R, tag="kT")
                for src, dst in ((qn, qT), (kn, kT)):
                    for g in range(N_KT // 4):
                        ptr = ps_t.tile([P, CHUNK], F32R, tag="ptr")
                        for t4 in range(4):
                            t = 4 * g + t4
                            nc.tensor.transpose(
                                ptr[:, t4 * P:(t4 + 1) * P], src[:, t, :], ident_r)
                        nc.vector.tensor_copy(
                            dst[:, g * CHUNK:(g + 1) * CHUNK], ptr)

                # ---- main loop over query chunks ----
                for c in range(N_CH):
                    jmax = 4 * c + 3
                    psum_o = ps_o.tile([P, CHUNK], F32, tag="po")
                    psum_d = ps_d.tile([1, CHUNK], F32, tag="pd")

                    for jp in range((jmax + 1 + 1) // 2):  # pairs of key tiles
                        j0 = 2 * jp
                        js = [j for j in (j0, j0 + 1) if j <= jmax]
                        psum_s = ps_s.tile([P, 2 * CHUNK], F32, tag="psm")
                        pexp = px.tile([P, 2 * CHUNK], F32R, tag="pexp")

                        offs = []
                        for j in js:
                            oj = max(0, P * j - CHUNK * c)
                            base = (j - j0) * CHUNK
                            offs.append((j, oj, base))
                            nc.tensor.matmul(
                                psum_s[:, base + oj:base + CHUNK],
                                kT[:, j * P:(j + 1) * P],
                                qT[:, CHUNK * c + oj:CHUNK * (c + 1)],
                                start=True, stop=True)

                        # exp (+ causal masking of diagonal 128-blocks)
                        diag = [(j, oj, base) for (j, oj, base) in offs
                                if j * P >= CHUNK * c]
                        if not diag:
                            nc.scalar.activation(
                                pexp[:, 0:len(js) * CHUNK],
                                psum_s[:, 0:len(js) * CHUNK],
                                EXP, scale=1.0 / TEMPERATURE)
                        else:
                            for (j, oj, base) in offs:
                                if j * P < CHUNK * c:  # fully below diagonal
                                    nc.scalar.activation(
                                        pexp[:, base:base + CHUNK],
                                        psum_s[:, base:base + CHUNK],
                                        EXP, scale=1.0 / TEMPERATURE)
                                else:
                                    # straight part after the triangular block
                                    if oj + P < CHUNK:
                                        nc.scalar.activation(
                                            pexp[:, base + oj + P:base + CHUNK],
                                            psum_s[:, base + oj + P:base + CHUNK],
                                            EXP, scale=1.0 / TEMPERATURE)
                                    # triangular block -> scratch, mask into pexp
                                    scr = sm.tile([P, P], F32R, tag="scr")
                                    nc.scalar.activation(
                                        scr, psum_s[:, base + oj:base + oj + P],
                                        EXP, scale=1.0 / TEMPERATURE)
                                    nc.gpsimd.tensor_mul(
                                        pexp[:, base + oj:base + oj + P], scr, utm)

                        # PV and denominator accumulation
                        for (j, oj, base) in offs:
                            nc.tensor.matmul(
                                psum_o[:, oj:CHUNK],
                                vn[:, j, :],
                                pexp[:, base + oj:base + CHUNK],
                                start=(j == 0), stop=(j == jmax),
                                skip_group_check=True)
                            nc.tensor.matmul(
                                psum_d[:, oj:CHUNK],
                                ones_col,
                                pexp[:, base + oj:base + CHUNK],
                                start=(j == 0), stop=(j == jmax),
                                skip_group_check=True)

                    # ---- normalize + transpose + store ----
                    den = sm.tile([1, CHUNK], F32, tag="den")
                    nc.vector.tensor_copy(den, psum_d)
                    rec = sm.tile([1, CHUNK], F32, tag="rec")
                    nc.vector.reciprocal_approx_fast(rec, den)
                    bc = sm.tile([P, CHUNK], F32, tag="bc")
                    nc.gpsimd.partition_broadcast(bc, rec)
                    outn = sm.tile([P, CHUNK], F32, tag="outn")
                    nc.vector.tensor_mul(outn, psum_o, bc)

                    ptr2 = ps_t.tile([P, CHUNK], F32, tag="ptr")
                    for tt in range(4):
                        nc.tensor.transpose(
                            ptr2[:, tt * P:(tt + 1) * P],
                            outn[:, tt * P:(tt + 1) * P], ident)
                    outT = sm.tile([P, 4, P], F32, tag="outT")
                    nc.vector.tensor_copy(outT, ptr2)
                    nc.sync.dma_start(
                        out=o_d[h, CHUNK * c:CHUNK * (c + 1), :].rearrange(
                            "(t p) d -> p t d", p=P),
                        in_=outT)

    nc.compile()
    return nc


_NC_CACHE = None


def _get_nc():
    global _NC_CACHE
    if _NC_CACHE is None:
        _NC_CACHE = build_attention_nc()
    return _NC_CACHE


def kernel(q, k, v, mask=None, _trace=False):
    """Full-input entry point: q,k,v [2,16,2048,128] f32, mask [2,1,2048,2048]
    int32 (causal; the kernel hardcodes causality and does not read it).
    Returns [2,16,2048,128] f32."""
    nc = _get_nc()
    qf = np.ascontiguousarray(np.asarray(q, dtype=np.float32).reshape(B * H, S, D))
    kf = np.ascontiguousarray(np.asarray(k, dtype=np.float32).reshape(B * H, S, D))
    vf = np.ascontiguousarray(np.asarray(v, dtype=np.float32).reshape(B * H, S, D))
    in_maps = []
    for i in range(N_CORES):
        sl = slice(i * HEADS_PER_CORE, (i + 1) * HEADS_PER_CORE)
        in_maps.append({"q": qf[sl], "k": kf[sl], "v": vf[sl]})
    res = run_bass_kernel_spmd(nc, in_maps, list(range(N_CORES)), trace=_trace)
    out = np.concatenate([res.results[i]["out"] for i in range(N_CORES)], axis=0)
    out = out.reshape(B, H, S, D).astype(np.float32)
    if _trace:
        return out, res
    return out


# revision 5
# speedup vs baseline: 1.0468x; 1.0468x over previous
"""Causal scaled-dot-product attention for Trainium2 (Bass/Tile), 8-core SPMD.

Problem: B=2, H=16, S=2048, D=128 fp32, causal mask, softmax(QK^T/sqrt(D)) @ V.
Sharding: batch*heads (32) split across 8 cores, 4 heads per core. Attention is
independent per (b,h): no communication.

Per-head algorithm (S^T layout — avoids any transpose of the probability
matrix):
  - PE-transpose Q,K once -> Q^T,K^T  [d=128 partitions, seq free]
  - for each 512-wide query chunk c:
      for each key tile j (128 keys) at or below the diagonal:
        S^T[j] = K_j @ Q_c^T          (fp32r matmul, PSUM)
        P^T[j] = exp(S^T[j] / temp)   (ACT, PSUM->SBUF, f32r)
        diagonal tiles masked with an upper-triangular constant
        OUT^T  += V_j^T @ P^T[j]      (fp32r matmul, V in natural layout)
        den    += ones^T @ P^T[j]     (fp32r matmul, [1, 512])
      OUT = transpose(OUT^T * (1/den)) -> DRAM
Softmax max-subtraction is skipped: logits are bounded (~20) so exp is safe in
fp32, and softmax is shift-invariant.

Emission is software-pipelined so the in-order PE never waits: PV/den matmuls
for group g are emitted after group g+1's QK/exp; chunk tails are deferred two
groups; the next head's load + Q/K transposes are interleaved into the current
head's main loop.
"""
from collections import deque

import numpy as np

import concourse.bacc as bacc
import concourse.tile as tile
import concourse.mybir as mybir
from concourse.bass_utils import run_bass_kernel_spmd
from concourse.masks import make_identity, make_upper_triangular

F32 = mybir.dt.float32
F32R = mybir.dt.float32r
EXP = mybir.ActivationFunctionType.Exp

B, H, S, D = 2, 16, 2048, 128
TEMPERATURE = 11.313708498984761  # sqrt(128)
N_CORES = 8
HEADS_PER_CORE = (B * H) // N_CORES  # 4
P = 128                    # partitions / tile edge
CHUNK = 512                # query chunk (1 PSUM bank of fp32)
N_KT = S // P              # 16 key tiles per head
N_CH = S // CHUNK          # 4 query chunks per head


def build_attention_nc(rep=1):
    nc = bacc.Bacc("TRN2", target_bir_lowering=False, debug=False,
                   num_devices=N_CORES)
    q_d = nc.dram_tensor("q", [HEADS_PER_CORE, S, D], F32, kind="ExternalInput").ap()
    k_d = nc.dram_tensor("k", [HEADS_PER_CORE, S, D], F32, kind="ExternalInput").ap()
    v_d = nc.dram_tensor("v", [HEADS_PER_CORE, S, D], F32, kind="ExternalInput").ap()
    o_d = nc.dram_tensor("out", [HEADS_PER_CORE, S, D], F32, kind="ExternalOutput").ap()

    n_heads = rep * HEADS_PER_CORE

    with tile.TileContext(nc) as tc:
        with tc.tile_pool(name="consts", bufs=1) as consts, \
             tc.tile_pool(name="inb", bufs=2) as inb, \
             tc.tile_pool(name="qkt", bufs=2) as qkt, \
             tc.tile_pool(name="px", bufs=4) as px, \
             tc.tile_pool(name="sm", bufs=3) as sm, \
             tc.tile_pool(name="ps_s", bufs=2, space="PSUM") as ps_s, \
             tc.tile_pool(name="ps_o", bufs=2, space="PSUM") as ps_o, \
             tc.tile_pool(name="ps_d", bufs=1, space="PSUM") as ps_d, \
             tc.tile_pool(name="ps_t", bufs=1, space="PSUM") as ps_t:

            # ---- constants ----
            ident = consts.tile([P, P], F32)
            make_identity(nc, ident)
            ident_r = consts.tile([P, P], F32R)
            nc.vector.tensor_copy(ident_r, ident)
            utm = consts.tile([P, P], F32)  # utm[k,q] = 1 iff q >= k
            make_upper_triangular(nc, utm, val=1.0, diag=True)
            ones_f = consts.tile([P, 1], F32)
            nc.vector.memset(ones_f, 1.0)
            ones_col = consts.tile([P, 1], F32R)
            nc.vector.tensor_copy(ones_col, ones_f)

            head_state = {}

            def emit_load(hh):
                h = hh % HEADS_PER_CORE
                qn = inb.tile([P, N_KT, P], F32R, tag="qn", name="qn")
                kn = inb.tile([P, N_KT, P], F32R, tag="kn", name="kn")
                vn = inb.tile([P, N_KT, P], F32R, tag="vn", name="vn")
                nc.gpsimd.dma_start(
                    out=qn, in_=q_d[h].rearrange("(t p) d -> p t d", p=P))
                nc.gpsimd.dma_start(
                    out=kn, in_=k_d[h].rearrange("(t p) d -> p t d", p=P))
                nc.gpsimd.dma_start(
                    out=vn, in_=v_d[h].rearrange("(t p) d -> p t d", p=P))
                qT = qkt.tile([P, S], F32R, tag="qT", name="qT")
                kT = qkt.tile([P, S], F32R, tag="kT", name="kT")
                head_state[hh] = dict(qn=qn, kn=kn, vn=vn, qT=qT, kT=kT)

            def prep_tasks(hh):
                """Closures: transpose 4 tiles of Q or K -> qT/kT, per group."""
                tasks = []
                for src_key, dst_key in (("qn", "qT"), ("kn", "kT")):
                    for g in range(N_KT // 4):
                        def t(src_key=src_key, dst_key=dst_key, g=g, hh=hh):
                            st = head_state[hh]
                            src, dst = st[src_key], st[dst_key]
                            ptr = ps_t.tile([P, CHUNK], F32R, tag="ptr",
                                            name="ptr")
                            for t4 in range(4):
                                tt = 4 * g + t4
                                nc.tensor.transpose(
                                    ptr[:, t4 * P:(t4 + 1) * P],
                                    src[:, tt, :], ident_r)
                            nc.vector.tensor_copy(
                                dst[:, g * CHUNK:(g + 1) * CHUNK], ptr)
                        tasks.append(t)
                return tasks

            def make_pv(st, offs, pexp, psum_o, psum_d, jmax):
                def emit():
                    for (j, oj, base) in offs:
                        nc.tensor.matmul(
                            psum_o[:, oj:CHUNK], st["vn"][:, j, :],
                            pexp[:, base + oj:base + CHUNK],
                            start=(j == 0), stop=(j == jmax),
                            skip_group_check=True)
                        nc.tensor.matmul(
                            psum_d[:, oj:CHUNK], ones_col,
                            pexp[:, base + oj:base + CHUNK],
                            start=(j == 0), stop=(j == jmax),
                            skip_group_check=True)
                return emit

            def make_tail(hh, c, psum_o, psum_d):
                def emit():
                    h = hh % HEADS_PER_CORE
                    den = sm.tile([1, CHUNK], F32, tag="den", name="den")
                    nc.vector.tensor_copy(den, psum_d)
                    rec = sm.tile([1, CHUNK], F32, tag="rec", name="rec")
                    nc.vector.reciprocal_approx_fast(rec, den)
                    bc = sm.tile([P, CHUNK], F32, tag="bc", name="bc")
                    nc.gpsimd.partition_broadcast(bc, rec)
                    outn = sm.tile([P, CHUNK], F32, tag="outn", name="outn")
                    nc.vector.tensor_mul(outn, psum_o, bc)
                    ptr2 = ps_t.tile([P, CHUNK], F32, tag="ptr", name="ptr")
                    for tt in range(4):
                        nc.tensor.transpose(
                            ptr2[:, tt * P:(tt + 1) * P],
                            outn[:, tt * P:(tt + 1) * P], ident)
                    outT = sm.tile([P, 4, P], F32, tag="outT", name="outT")
                    nc.vector.tensor_copy(outT, ptr2)
                    nc.sync.dma_start(
                        out=o_d[h, CHUNK * c:CHUNK * (c + 1), :].rearrange(
                            "(t p) d -> p t d", p=P),
                        in_=outT)
                return emit

            # head 0: load + prep upfront (cannot be hidden)
            emit_load(0)
            for t in prep_tasks(0):
                t()

            for hh in range(n_heads):
                st = head_state[hh]
                if hh + 1 < n_heads:
                    emit_load(hh + 1)
                    pending_prep = deque(prep_tasks(hh + 1))
                else:
                    pending_prep = deque()

                pending_pv = None          # PV/den of previous group
                deferred = []              # [(age, closure)] chunk tails
                group_idx = 0

                def after_group(pending_prep=pending_prep, deferred=deferred):
                    # emit one prep task for the next head every other group,
                    # and any tail that has aged >= 2 groups
                    for item in list(deferred):
                        if group_idx - item[0] >= 2:
                            item[1]()
                            deferred.remove(item)

                for c in range(N_CH):
                    jmax = 4 * c + 3
                    psum_o = ps_o.tile([P, CHUNK], F32, tag="po", name="po")
                    psum_d = ps_d.tile([1, CHUNK], F32, tag="pd", name="pd")

                    for jp in range((jmax + 2) // 2):
                        j0 = 2 * jp
                        js = [j for j in (j0, j0 + 1) if j <= jmax]
                        psum_s = ps_s.tile([P, 2 * CHUNK], F32, tag="psm",
                                           name="psm")
                        pexp = px.tile([P, 2 * CHUNK], F32R, tag="pexp",
                                       name="pexp")

                        offs = []
                        for j in js:
                            oj = max(0, P * j - CHUNK * c)
                            base = (j - j0) * CHUNK
                            offs.append((j, oj, base))
                            nc.tensor.matmul(
                                psum_s[:, base + oj:base + CHUNK],
                                st["kT"][:, j * P:(j + 1) * P],
                                st["qT"][:, CHUNK * c + oj:CHUNK * (c + 1)],
                                start=True, stop=True)

                        # exp (+ causal masking of diagonal 128-blocks)
                        diag = any(j * P >= CHUNK * c for (j, oj, base) in offs)
                        if not diag:
                            nc.scalar.activation(
                                pexp[:, 0:len(js) * CHUNK],
                                psum_s[:, 0:len(js) * CHUNK],
                                EXP, scale=1.0 / TEMPERATURE)
                        else:
                            for (j, oj, base) in offs:
                                if j * P < CHUNK * c:
                                    nc.scalar.activation(
                                        pexp[:, base:base + CHUNK],
                                        psum_s[:, base:base + CHUNK],
                                        EXP, scale=1.0 / TEMPERATURE)
                                else:
                                    if oj + P < CHUNK:
                                        nc.scalar.activation(
                                            pexp[:, base + oj + P:base + CHUNK],
                                            psum_s[:, base + oj + P:base + CHUNK],
                                            EXP, scale=1.0 / TEMPERATURE)
                                    scr = sm.tile([P, P], F32R, tag="scr",
                                                  name="scr")
                                    nc.scalar.activation(
                                        scr, psum_s[:, base + oj:base + oj + P],
                                        EXP, scale=1.0 / TEMPERATURE)
                                    nc.gpsimd.tensor_mul(
                                        pexp[:, base + oj:base + oj + P],
                                        scr, utm)

                        if pending_pv is not None:
                            pending_pv()
                        pending_pv = make_pv(st, offs, pexp, psum_o, psum_d,
                                             jmax)

                        group_idx += 1
                        if pending_prep and group_idx % 2 == 0:
                            pending_prep.popleft()()
                        after_group()

                    deferred.append((group_idx, make_tail(hh, c, psum_o,
                                                          psum_d)))

                # flush this head
                if pending_pv is not None:
                    pending_pv()
                while pending_prep:
                    pending_prep.popleft()()
                for item in deferred:
                    item[1]()

    nc.compile()
    return nc


_NC_CACHE = None


def _get_nc():
    global _NC_CACHE
    if _NC_CACHE is None:
        _NC_CACHE = build_attention_nc()
    return _NC_CACHE


def kernel(q, k, v, mask=None, _trace=False):
    """Full-input entry point: q,k,v [2,16,2048,128] f32, mask [2,1,2048,2048]
    int32 (causal; the kernel hardcodes causality and does not read it).
    Returns [2,16,2048,128] f32."""
    nc = _get_nc()
    qf = np.ascontiguousarray(np.asarray(q, dtype=np.float32).reshape(B * H, S, D))
    kf = np.ascontiguousarray(np.asarray(k, dtype=np.float32).reshape(B * H, S, D))
    vf = np.ascontiguousarray(np.asarray(v, dtype=np.float32).reshape(B * H, S, D))
    in_maps = []
    for i in range(N_CORES):
        sl = slice(i * HEADS_PER_CORE, (i + 1) * HEADS_PER_CORE)
        in_maps.append({"q": qf[sl], "k": kf[sl], "v": vf[sl]})
    res = run_bass_kernel_spmd(nc, in_maps, list(range(N_CORES)), trace=_trace)
    out = np.concatenate([res.results[i]["out"] for i in range(N_CORES)], axis=0)
    out = out.reshape(B, H, S, D).astype(np.float32)
    if _trace:
        return out, res
    return out


# revision 8
# speedup vs baseline: 1.9983x; 1.9089x over previous
"""Causal scaled-dot-product attention for Trainium2 (Bass/Tile), 8-core SPMD.

Problem: B=2, H=16, S=2048, D=128 fp32, causal mask, softmax(QK^T/sqrt(D)) @ V.
Sharding: batch*heads (32) split across 8 cores, 4 heads per core. Attention is
independent per (b,h): no communication.

Per-head algorithm (S^T layout — avoids any transpose of the probability
matrix):
  - PE-transpose Q,K once -> Q^T,K^T  [d=128 partitions, seq free]
  - for each 512-wide query chunk c:
      for each key tile j (128 keys) at or below the diagonal:
        S^T[j] = K_j @ Q_c^T          (fp32r matmul, PSUM)
        P^T[j] = exp(S^T[j] / temp)   (ACT, PSUM->SBUF, f32r)
        diagonal tiles masked with an upper-triangular constant
        OUT^T  += V_j^T @ P^T[j]      (fp32r matmul, V in natural layout)
        den    += ones^T @ P^T[j]     (fp32r matmul, [1, 512])
      OUT = transpose(OUT^T * (1/den)) -> DRAM
Softmax max-subtraction is skipped: logits are bounded (~20) so exp is safe in
fp32, and softmax is shift-invariant.

Emission is software-pipelined so the in-order PE never waits: PV/den matmuls
for group g are emitted after group g+1's QK/exp; chunk tails are deferred two
groups; the next head's load + Q/K transposes are interleaved into the current
head's main loop.
"""
from collections import deque

import numpy as np

import concourse.bacc as bacc
import concourse.tile as tile
import concourse.mybir as mybir
from concourse.bass_utils import run_bass_kernel_spmd
from concourse.masks import make_identity, make_upper_triangular

F32 = mybir.dt.float32
F32R = mybir.dt.float32r
EXP = mybir.ActivationFunctionType.Exp

B, H, S, D = 2, 16, 2048, 128
TEMPERATURE = 11.313708498984761  # sqrt(128)
N_CORES = 8
HEADS_PER_CORE = (B * H) // N_CORES  # 4
P = 128                    # partitions / tile edge
CHUNK = 512                # query chunk (1 PSUM bank of fp32)
N_KT = S // P              # 16 key tiles per head
N_CH = S // CHUNK          # 4 query chunks per head


def build_attention_nc(rep=1):
    nc = bacc.Bacc("TRN2", target_bir_lowering=False, debug=False,
                   num_devices=N_CORES)
    q_d = nc.dram_tensor("q", [HEADS_PER_CORE, S, D], F32, kind="ExternalInput").ap()
    k_d = nc.dram_tensor("k", [HEADS_PER_CORE, S, D], F32, kind="ExternalInput").ap()
    v_d = nc.dram_tensor("v", [HEADS_PER_CORE, S, D], F32, kind="ExternalInput").ap()
    o_d = nc.dram_tensor("out", [HEADS_PER_CORE, S, D], F32, kind="ExternalOutput").ap()

    n_heads = rep * HEADS_PER_CORE

    with tile.TileContext(nc) as tc:
        with tc.tile_pool(name="consts", bufs=1) as consts, \
             tc.tile_pool(name="inb", bufs=2) as inb, \
             tc.tile_pool(name="qkt", bufs=2) as qkt, \
             tc.tile_pool(name="px", bufs=4) as px, \
             tc.tile_pool(name="sm", bufs=3) as sm, \
             tc.tile_pool(name="ps_s", bufs=2, space="PSUM") as ps_s, \
             tc.tile_pool(name="ps_o", bufs=2, space="PSUM") as ps_o, \
             tc.tile_pool(name="ps_d", bufs=1, space="PSUM") as ps_d, \
             tc.tile_pool(name="ps_t", bufs=1, space="PSUM") as ps_t:

            # ---- constants ----
            ident = consts.tile([P, P], F32)
            make_identity(nc, ident)
            utm = consts.tile([P, P], F32)  # utm[k,q] = 1 iff q >= k
            make_upper_triangular(nc, utm, val=1.0, diag=True)
            ones_f = consts.tile([P, 1], F32)
            nc.vector.memset(ones_f, 1.0)
            ones_col = consts.tile([P, 1], F32R)
            nc.vector.tensor_copy(ones_col, ones_f)

            head_state = {}

            def emit_load(hh):
                h = hh % HEADS_PER_CORE
                qn = inb.tile([P, N_KT, P], F32, tag="qn", name="qn")
                kn = inb.tile([P, N_KT, P], F32, tag="kn", name="kn")
                vn = inb.tile([P, N_KT, P], F32, tag="vn", name="vn")
                nc.sync.dma_start(
                    out=qn, in_=q_d[h].rearrange("(t p) d -> p t d", p=P))
                nc.sync.dma_start(
                    out=kn, in_=k_d[h].rearrange("(t p) d -> p t d", p=P))
                nc.sync.dma_start(
                    out=vn, in_=v_d[h].rearrange("(t p) d -> p t d", p=P))
                qT = qkt.tile([P, S], F32R, tag="qT", name="qT")
                kT = qkt.tile([P, S], F32R, tag="kT", name="kT")
                vnr = qkt.tile([P, N_KT, P], F32R, tag="vnr", name="vnr")
                head_state[hh] = dict(qn=qn, kn=kn, vn=vn, qT=qT, kT=kT,
                                      vnr=vnr)

            def prep_tasks(hh):
                """Closures: transpose 4 tiles of Q or K -> qT/kT per group,
                plus cast V -> f32r."""
                tasks = []
                for src_key, dst_key in (("qn", "qT"), ("kn", "kT")):
                    for g in range(N_KT // 4):
                        def t(src_key=src_key, dst_key=dst_key, g=g, hh=hh):
                            st = head_state[hh]
                            src, dst = st[src_key], st[dst_key]
                            ptr = ps_t.tile([P, CHUNK], F32, tag="ptr",
                                            name="ptr")
                            for t4 in range(4):
                                tt = 4 * g + t4
                                nc.tensor.transpose(
                                    ptr[:, t4 * P:(t4 + 1) * P],
                                    src[:, tt, :], ident)
                            nc.vector.tensor_copy(
                                dst[:, g * CHUNK:(g + 1) * CHUNK], ptr)
                        tasks.append(t)

                def tv(hh=hh):
                    st = head_state[hh]
                    nc.vector.tensor_copy(st["vnr"], st["vn"])
                tasks.append(tv)
                return tasks

            def make_pv(st, offs, pexp, psum_o, psum_d, jmax):
                def emit():
                    for (j, oj, base) in offs:
                        nc.tensor.matmul(
                            psum_o[:, oj:CHUNK], st["vnr"][:, j, :],
                            pexp[:, base + oj:base + CHUNK],
                            start=(j == 0), stop=(j == jmax),
                            skip_group_check=True)
                        nc.tensor.matmul(
                            psum_d[:, oj:CHUNK], ones_col,
                            pexp[:, base + oj:base + CHUNK],
                            start=(j == 0), stop=(j == jmax),
                            skip_group_check=True)
                return emit

            def make_tail(hh, c, psum_o, psum_d):
                def emit():
                    h = hh % HEADS_PER_CORE
                    # evacuate OUT^T immediately (independent of denominators)
                    outn = sm.tile([P, CHUNK], F32, tag="outn", name="outn")
                    nc.vector.tensor_copy(outn, psum_o)
                    # move denominators onto row 0 of a padded tile (rows
                    # 1..127 are never consumed), transpose to per-q columns
                    pad = sm.tile([P, CHUNK], F32, tag="pad", name="pad")
                    nc.vector.tensor_copy(pad[0:1, :], psum_d)
                    for tt in range(4):
                        nc.tensor.transpose(
                            psum_o[:, tt * P:(tt + 1) * P],
                            pad[:, tt * P:(tt + 1) * P], ident)
                    den4 = sm.tile([P, 4], F32, tag="den4", name="den4")
                    nc.vector.tensor_copy(
                        den4,
                        psum_o.rearrange("p (a b) -> p a b", b=P)[:, :, 0])
                    rc4 = sm.tile([P, 4], F32, tag="rc4", name="rc4")
                    nc.vector.reciprocal_approx_fast(rc4, den4)
                    # transpose OUT^T back to [q, d]
                    ptr2 = ps_t.tile([P, CHUNK], F32, tag="ptr", name="ptr")
                    for tt in range(4):
                        nc.tensor.transpose(
                            ptr2[:, tt * P:(tt + 1) * P],
                            outn[:, tt * P:(tt + 1) * P], ident)
                    # normalize during the final evacuation
                    outT = sm.tile([P, 4, P], F32, tag="outT", name="outT")
                    for tt in range(4):
                        nc.vector.tensor_scalar_mul(
                            outT[:, tt, :], ptr2[:, tt * P:(tt + 1) * P],
                            rc4[:, tt:tt + 1])
                    nc.sync.dma_start(
                        out=o_d[h, CHUNK * c:CHUNK * (c + 1), :].rearrange(
                            "(t p) d -> p t d", p=P),
                        in_=outT)
                return emit

            # head 0: load + prep upfront (cannot be hidden)
            emit_load(0)
            for t in prep_tasks(0):
                t()

            for hh in range(n_heads):
                st = head_state[hh]
                if hh + 1 < n_heads:
                    emit_load(hh + 1)
                    pending_prep = deque(prep_tasks(hh + 1))
                else:
                    pending_prep = deque()

                pending_pv = None          # PV/den of previous group
                deferred = []              # [(age, closure)] chunk tails
                group_idx = 0

                def after_group(pending_prep=pending_prep, deferred=deferred):
                    # emit one prep task for the next head every other group,
                    # and any tail that has aged >= 2 groups
                    for item in list(deferred):
                        if group_idx - item[0] >= 2:
                            item[1]()
                            deferred.remove(item)

                for c in range(N_CH):
                    jmax = 4 * c + 3
                    psum_o = ps_o.tile([P, CHUNK], F32, tag="po", name="po")
                    psum_d = ps_d.tile([1, CHUNK], F32, tag="pd", name="pd")

                    for jp in range((jmax + 2) // 2):
                        j0 = 2 * jp
                        js = [j for j in (j0, j0 + 1) if j <= jmax]
                        psum_s = ps_s.tile([P, 2 * CHUNK], F32, tag="psm",
                                           name="psm")
                        pexp = px.tile([P, 2 * CHUNK], F32R, tag="pexp",
                                       name="pexp")

                        offs = []
                        for j in js:
                            oj = max(0, P * j - CHUNK * c)
                            base = (j - j0) * CHUNK
                            offs.append((j, oj, base))
                            nc.tensor.matmul(
                                psum_s[:, base + oj:base + CHUNK],
                                st["kT"][:, j * P:(j + 1) * P],
                                st["qT"][:, CHUNK * c + oj:CHUNK * (c + 1)],
                                start=True, stop=True)

                        # exp (+ causal masking of diagonal 128-blocks)
                        diag = any(j * P >= CHUNK * c for (j, oj, base) in offs)
                        if not diag:
                            nc.scalar.activation(
                                pexp[:, 0:len(js) * CHUNK],
                                psum_s[:, 0:len(js) * CHUNK],
                                EXP, scale=1.0 / TEMPERATURE)
                        else:
                            for (j, oj, base) in offs:
                                if j * P < CHUNK * c:
                                    nc.scalar.activation(
                                        pexp[:, base:base + CHUNK],
                                        psum_s[:, base:base + CHUNK],
                                        EXP, scale=1.0 / TEMPERATURE)
                                else:
                                    if oj + P < CHUNK:
                                        nc.scalar.activation(
                                            pexp[:, base + oj + P:base + CHUNK],
                                            psum_s[:, base + oj + P:base + CHUNK],
                                            EXP, scale=1.0 / TEMPERATURE)
                                    scr = sm.tile([P, P], F32R, tag="scr",
                                                  name="scr")
                                    nc.scalar.activation(
                                        scr, psum_s[:, base + oj:base + oj + P],
                                        EXP, scale=1.0 / TEMPERATURE)
                                    nc.gpsimd.tensor_mul(
                                        pexp[:, base + oj:base + oj + P],
                                        scr, utm)

                        if pending_pv is not None:
                            pending_pv()
                        pending_pv = make_pv(st, offs, pexp, psum_o, psum_d,
                                             jmax)

                        group_idx += 1
                        if pending_prep and group_idx % 2 == 0:
                            pending_prep.popleft()()
                        after_group()

                    deferred.append((group_idx, make_tail(hh, c, psum_o,
                                                          psum_d)))

                # flush this head
                if pending_pv is not None:
                    pending_pv()
                while pending_prep:
                    pending_prep.popleft()()
                for item in deferred:
                    item[1]()

    nc.compile()
    return nc


_NC_CACHE = None


def _get_nc():
    global _NC_CACHE
    if _NC_CACHE is None:
        _NC_CACHE = build_attention_nc()
    return _NC_CACHE


def kernel(q, k, v, mask=None, _trace=False):
    """Full-input entry point: q,k,v [2,16,2048,128] f32, mask [2,1,2048,2048]
    int32 (causal; the kernel hardcodes causality and does not read it).
    Returns [2,16,2048,128] f32."""
    nc = _get_nc()
    qf = np.ascontiguousarray(np.asarray(q, dtype=np.float32).reshape(B * H, S, D))
    kf = np.ascontiguousarray(np.asarray(k, dtype=np.float32).reshape(B * H, S, D))
    vf = np.ascontiguousarray(np.asarray(v, dtype=np.float32).reshape(B * H, S, D))
    in_maps = []
    for i in range(N_CORES):
        sl = slice(i * HEADS_PER_CORE, (i + 1) * HEADS_PER_CORE)
        in_maps.append({"q": qf[sl], "k": kf[sl], "v": vf[sl]})
    res = run_bass_kernel_spmd(nc, in_maps, list(range(N_CORES)), trace=_trace)
    out = np.concatenate([res.results[i]["out"] for i in range(N_CORES)], axis=0)
    out = out.reshape(B, H, S, D).astype(np.float32)
    if _trace:
        return out, res
    return out


# revision 10
# speedup vs baseline: 2.0090x; 1.0053x over previous
"""Causal scaled-dot-product attention for Trainium2 (Bass/Tile), 8-core SPMD.

Problem: B=2, H=16, S=2048, D=128 fp32, causal mask, softmax(QK^T/sqrt(D)) @ V.
Sharding: batch*heads (32) split across 8 cores, 4 heads per core. Attention is
independent per (b,h): no communication.

Per-head algorithm (S^T layout — avoids any transpose of the probability
matrix):
  - PE-transpose Q,K once -> Q^T,K^T  [d=128 partitions, seq free]
  - for each 512-wide query chunk c:
      for each key tile j (128 keys) at or below the diagonal:
        S^T[j] = K_j @ Q_c^T          (fp32r matmul, PSUM)
        P^T[j] = exp(S^T[j] / temp)   (ACT, PSUM->SBUF, f32r)
        diagonal tiles masked with an upper-triangular constant
        OUT^T  += V_j^T @ P^T[j]      (fp32r matmul, V in natural layout)
        den    += ones^T @ P^T[j]     (fp32r matmul, [1, 512])
      OUT = transpose(OUT^T * (1/den)) -> DRAM
Softmax max-subtraction is skipped: logits are bounded (~20) so exp is safe in
fp32, and softmax is shift-invariant.

Emission is software-pipelined so the in-order PE never waits: PV/den matmuls
for group g are emitted after group g+1's QK/exp; chunk tails are deferred two
groups; the next head's load + Q/K transposes are interleaved into the current
head's main loop.
"""
from collections import deque

import numpy as np

import concourse.bacc as bacc
import concourse.tile as tile
import concourse.mybir as mybir
from concourse.bass_utils import run_bass_kernel_spmd
from concourse.masks import make_identity, make_upper_triangular

F32 = mybir.dt.float32
F32R = mybir.dt.float32r
EXP = mybir.ActivationFunctionType.Exp

B, H, S, D = 2, 16, 2048, 128
TEMPERATURE = 11.313708498984761  # sqrt(128)
N_CORES = 8
HEADS_PER_CORE = (B * H) // N_CORES  # 4
P = 128                    # partitions / tile edge
CHUNK = 512                # query chunk (1 PSUM bank of fp32)
N_KT = S // P              # 16 key tiles per head
N_CH = S // CHUNK          # 4 query chunks per head


def build_attention_nc(rep=1):
    nc = bacc.Bacc("TRN2", target_bir_lowering=False, debug=False,
                   num_devices=N_CORES)
    q_d = nc.dram_tensor("q", [HEADS_PER_CORE, S, D], F32, kind="ExternalInput").ap()
    k_d = nc.dram_tensor("k", [HEADS_PER_CORE, S, D], F32, kind="ExternalInput").ap()
    v_d = nc.dram_tensor("v", [HEADS_PER_CORE, S, D], F32, kind="ExternalInput").ap()
    o_d = nc.dram_tensor("out", [HEADS_PER_CORE, S, D], F32, kind="ExternalOutput").ap()

    n_heads = rep * HEADS_PER_CORE

    with tile.TileContext(nc) as tc:
        with tc.tile_pool(name="consts", bufs=1) as consts, \
             tc.tile_pool(name="inb", bufs=2) as inb, \
             tc.tile_pool(name="qkt", bufs=2) as qkt, \
             tc.tile_pool(name="px", bufs=6) as px, \
             tc.tile_pool(name="sm", bufs=4) as sm, \
             tc.tile_pool(name="ps_s", bufs=2, space="PSUM") as ps_s, \
             tc.tile_pool(name="ps_o", bufs=2, space="PSUM") as ps_o, \
             tc.tile_pool(name="ps_d", bufs=1, space="PSUM") as ps_d, \
             tc.tile_pool(name="ps_t", bufs=1, space="PSUM") as ps_t:

            # ---- constants ----
            ident = consts.tile([P, P], F32)
            make_identity(nc, ident)
            utm = consts.tile([P, P], F32)  # utm[k,q] = 1 iff q >= k
            make_upper_triangular(nc, utm, val=1.0, diag=True)
            ones_f = consts.tile([P, 1], F32)
            nc.vector.memset(ones_f, 1.0)
            ones_col = consts.tile([P, 1], F32R)
            nc.vector.tensor_copy(ones_col, ones_f)

            head_state = {}

            def emit_load(hh):
                h = hh % HEADS_PER_CORE
                qn = inb.tile([P, N_KT, P], F32, tag="qn", name="qn")
                kn = inb.tile([P, N_KT, P], F32, tag="kn", name="kn")
                vn = inb.tile([P, N_KT, P], F32, tag="vn", name="vn")
                nc.sync.dma_start(
                    out=qn, in_=q_d[h].rearrange("(t p) d -> p t d", p=P))
                nc.sync.dma_start(
                    out=kn, in_=k_d[h].rearrange("(t p) d -> p t d", p=P))
                nc.sync.dma_start(
                    out=vn, in_=v_d[h].rearrange("(t p) d -> p t d", p=P))
                qT = qkt.tile([P, S], F32R, tag="qT", name="qT")
                kT = qkt.tile([P, S], F32R, tag="kT", name="kT")
                vnr = qkt.tile([P, N_KT, P], F32R, tag="vnr", name="vnr")
                head_state[hh] = dict(qn=qn, kn=kn, vn=vn, qT=qT, kT=kT,
                                      vnr=vnr)

            def prep_tasks(hh):
                """Closures: transpose 4 tiles of Q or K -> qT/kT per group,
                plus cast V -> f32r."""
                tasks = []
                for src_key, dst_key in (("qn", "qT"), ("kn", "kT")):
                    for g in range(N_KT // 4):
                        def t(src_key=src_key, dst_key=dst_key, g=g, hh=hh):
                            st = head_state[hh]
                            src, dst = st[src_key], st[dst_key]
                            ptr = ps_t.tile([P, CHUNK], F32, tag="ptr",
                                            name="ptr")
                            for t4 in range(4):
                                tt = 4 * g + t4
                                nc.tensor.transpose(
                                    ptr[:, t4 * P:(t4 + 1) * P],
                                    src[:, tt, :], ident)
                            nc.vector.tensor_copy(
                                dst[:, g * CHUNK:(g + 1) * CHUNK], ptr)
                        tasks.append(t)

                def tv(hh=hh):
                    st = head_state[hh]
                    nc.vector.tensor_copy(st["vnr"], st["vn"])
                tasks.append(tv)
                return tasks

            def make_pv(st, offs, pexp, psum_o, psum_d, jmax):
                def emit():
                    for (j, oj, base) in offs:
                        nc.tensor.matmul(
                            psum_o[:, oj:CHUNK], st["vnr"][:, j, :],
                            pexp[:, base + oj:base + CHUNK],
                            start=(j == 0), stop=(j == jmax),
                            skip_group_check=True)
                        nc.tensor.matmul(
                            psum_d[:, oj:CHUNK], ones_col,
                            pexp[:, base + oj:base + CHUNK],
                            start=(j == 0), stop=(j == jmax),
                            skip_group_check=True)
                return emit

            def make_tail(hh, c, psum_o, psum_d):
                def emit():
                    h = hh % HEADS_PER_CORE
                    # evacuate OUT^T immediately (independent of denominators)
                    outn = sm.tile([P, CHUNK], F32, tag="outn", name="outn")
                    nc.vector.tensor_copy(outn, psum_o)
                    # move denominators onto row 0 of a padded tile (rows
                    # 1..127 are never consumed), transpose to per-q columns
                    pad = sm.tile([P, CHUNK], F32, tag="pad", name="pad")
                    nc.vector.tensor_copy(pad[0:1, :], psum_d)
                    for tt in range(4):
                        nc.tensor.transpose(
                            psum_o[:, tt * P:(tt + 1) * P],
                            pad[:, tt * P:(tt + 1) * P], ident)
                    den4 = sm.tile([P, 4], F32, tag="den4", name="den4")
                    nc.vector.tensor_copy(
                        den4,
                        psum_o.rearrange("p (a b) -> p a b", b=P)[:, :, 0])
                    rc4 = sm.tile([P, 4], F32, tag="rc4", name="rc4")
                    nc.vector.reciprocal_approx_fast(rc4, den4)
                    # transpose OUT^T back to [q, d]
                    ptr2 = ps_t.tile([P, CHUNK], F32, tag="ptr", name="ptr")
                    for tt in range(4):
                        nc.tensor.transpose(
                            ptr2[:, tt * P:(tt + 1) * P],
                            outn[:, tt * P:(tt + 1) * P], ident)
                    # normalize during the final evacuation
                    outT = sm.tile([P, 4, P], F32, tag="outT", name="outT")
                    for tt in range(4):
                        nc.vector.tensor_scalar_mul(
                            outT[:, tt, :], ptr2[:, tt * P:(tt + 1) * P],
                            rc4[:, tt:tt + 1])
                    nc.sync.dma_start(
                        out=o_d[h, CHUNK * c:CHUNK * (c + 1), :].rearrange(
                            "(t p) d -> p t d", p=P),
                        in_=outT)
                return emit

            # head 0: load + prep upfront (cannot be hidden)
            emit_load(0)
            for t in prep_tasks(0):
                t()

            for hh in range(n_heads):
                st = head_state[hh]
                if hh + 1 < n_heads:
                    emit_load(hh + 1)
                    pending_prep = deque(prep_tasks(hh + 1))
                else:
                    pending_prep = deque()

                pending_pv = None          # PV/den of previous group
                deferred = []              # [(age, closure)] chunk tails
                group_idx = 0

                def after_group(pending_prep=pending_prep, deferred=deferred):
                    # emit one prep task for the next head every other group,
                    # and any tail that has aged >= 2 groups
                    for item in list(deferred):
                        if group_idx - item[0] >= 2:
                            item[1]()
                            deferred.remove(item)

                for c in range(N_CH):
                    jmax = 4 * c + 3
                    psum_o = ps_o.tile([P, CHUNK], F32, tag="po", name="po")
                    psum_d = ps_d.tile([1, CHUNK], F32, tag="pd", name="pd")

                    for jp in range((jmax + 2) // 2):
                        j0 = 2 * jp
                        js = [j for j in (j0, j0 + 1) if j <= jmax]
                        psum_s = ps_s.tile([P, 2 * CHUNK], F32, tag="psm",
                                           name="psm")
                        pexp = px.tile([P, 2 * CHUNK], F32R, tag="pexp",
                                       name="pexp")

                        offs = []
                        for j in js:
                            oj = max(0, P * j - CHUNK * c)
                            base = (j - j0) * CHUNK
                            offs.append((j, oj, base))
                            nc.tensor.matmul(
                                psum_s[:, base + oj:base + CHUNK],
                                st["kT"][:, j * P:(j + 1) * P],
                                st["qT"][:, CHUNK * c + oj:CHUNK * (c + 1)],
                                start=True, stop=True)

                        # exp (+ causal masking of diagonal 128-blocks,
                        # applied in place after the exp)
                        diag = any(j * P >= CHUNK * c for (j, oj, base) in offs)
                        if not diag:
                            nc.scalar.activation(
                                pexp[:, 0:len(js) * CHUNK],
                                psum_s[:, 0:len(js) * CHUNK],
                                EXP, scale=1.0 / TEMPERATURE)
                        else:
                            for (j, oj, base) in offs:
                                nc.scalar.activation(
                                    pexp[:, base + oj:base + CHUNK],
                                    psum_s[:, base + oj:base + CHUNK],
                                    EXP, scale=1.0 / TEMPERATURE)
                                if j * P >= CHUNK * c:
                                    nc.gpsimd.tensor_mul(
                                        pexp[:, base + oj:base + oj + P],
                                        pexp[:, base + oj:base + oj + P], utm)

                        if pending_pv is not None:
                            pending_pv()
                        pending_pv = make_pv(st, offs, pexp, psum_o, psum_d,
                                             jmax)

                        group_idx += 1
                        if pending_prep and group_idx % 2 == 0:
                            pending_prep.popleft()()
                        after_group()

                    deferred.append((group_idx, make_tail(hh, c, psum_o,
                                                          psum_d)))

                # flush this head
                if pending_pv is not None:
                    pending_pv()
                while pending_prep:
                    pending_prep.popleft()()
                for item in deferred:
                    item[1]()

    nc.compile()
    return nc


_NC_CACHE = None


def _get_nc():
    global _NC_CACHE
    if _NC_CACHE is None:
        _NC_CACHE = build_attention_nc()
    return _NC_CACHE


def kernel(q, k, v, mask=None, _trace=False):
    """Full-input entry point: q,k,v [2,16,2048,128] f32, mask [2,1,2048,2048]
    int32 (causal; the kernel hardcodes causality and does not read it).
    Returns [2,16,2048,128] f32."""
    nc = _get_nc()
    qf = np.ascontiguousarray(np.asarray(q, dtype=np.float32).reshape(B * H, S, D))
    kf = np.ascontiguousarray(np.asarray(k, dtype=np.float32).reshape(B * H, S, D))
    vf = np.ascontiguousarray(np.asarray(v, dtype=np.float32).reshape(B * H, S, D))
    in_maps = []
    for i in range(N_CORES):
        sl = slice(i * HEADS_PER_CORE, (i + 1) * HEADS_PER_CORE)
        in_maps.append({"q": qf[sl], "k": kf[sl], "v": vf[sl]})
    res = run_bass_kernel_spmd(nc, in_maps, list(range(N_CORES)), trace=_trace)
    out = np.concatenate([res.results[i]["out"] for i in range(N_CORES)], axis=0)
    out = out.reshape(B, H, S, D).astype(np.float32)
    if _trace:
        return out, res
    return out
